# revision 6
# baseline (speedup 1.0000x reference)
"""Trainium2 Bass kernel for differentiable NLMS (nn_DifferentiableNLMS).

Problem: B=32 batches, T=2000 frames, F=513 freq bins, L=32 filter taps.
Per (batch, freq) channel an independent sequential NLMS recurrence runs
along time:
    y_hat_k = sum_l W[l] * x[k-L+1+l]
    e_k     = y_k - y_hat_k
    n_k     = sum_l x[k-L+1+l]^2 + eps
    W      += mu * (e_k / n_k) * window_k

Sharding: batch dim across 8 cores (4 batches/core -> 2052 channels/core,
padded to 17*128).  Channels live on partitions (128) x free-dim blocks (17).
The norm reciprocals mu/n_k depend only on X and are precomputed host-side
in fp64 (cumsum sliding sums); the device executes the sequential recurrence
with 6 vector-engine ops per time step, reading the sliding window directly
out of the chunk-resident X tile via strided access patterns.
"""

import os
from contextlib import ExitStack

import numpy as np

import concourse.bacc as bacc
import concourse.bass as bass
import concourse.tile as tile
from concourse import mybir
from concourse.bass_utils import run_bass_kernel_spmd

L = 32
MU = 0.1
EPS = 1e-8
B, T_FULL, F = 32, 2000, 513
NCORES = 8
BPC = B // NCORES            # batches per core
C = BPC * F                  # 2052 channels per core
NBLK = (C + 127) // 128      # 17 free-dim blocks
CPAD = NBLK * 128            # 2176 padded channels
F32 = mybir.dt.float32

# module-level cache: (T, TC) -> (nc, names)
_PROGRAMS = {}
LAST_RESULTS = None


def _install_profiling_shims():
    """Best-effort NTFF profiling support under axon (missing antenv.axon_hooks)."""
    try:
        import contextlib
        import ctypes
        import sys
        import types

        import antenv  # noqa: F401
        import concourse.bass_utils as bu

        if not getattr(bu.upload_artifacts, "_nlms_patched", False):
            def _local_upload(tmpdir):
                return "local://" + str(tmpdir)

            _local_upload._nlms_patched = True
            bu.upload_artifacts = _local_upload

        try:
            import antenv.axon_hooks  # noqa: F401
            return
        except ImportError:
            pass

        so_path = os.environ.get("PJRT_LIBRARY_PATH", "/opt/axon/libaxon_pjrt.so")
        lib = ctypes.CDLL(so_path)
        if not hasattr(lib, "axon_start_nrt_profile"):
            return
        lib.axon_start_nrt_profile.argtypes = [
            ctypes.POINTER(ctypes.c_int64),
            ctypes.c_size_t,
        ]
        lib.axon_start_nrt_profile.restype = ctypes.c_int64
        lib.axon_stop_nrt_profile.argtypes = [ctypes.c_char_p]
        lib.axon_stop_nrt_profile.restype = ctypes.c_int64

        @contextlib.contextmanager
        def _hook(output_dir, device_ids):
            import jax

            jax.devices()
            if device_ids:
                ids = (ctypes.c_int64 * len(device_ids))(*device_ids)
                rc = lib.axon_start_nrt_profile(ids, len(device_ids))
            else:
                rc = lib.axon_start_nrt_profile(None, 0)
            if rc != 0:
                raise RuntimeError(f"axon_start_nrt_profile rc={rc}")
            try:
                yield
            finally:
                n = lib.axon_stop_nrt_profile(str(output_dir).encode())
                print(f"profile: {n} file(s) written to {output_dir}")

        mod = types.ModuleType("antenv.axon_hooks")
        mod.get_axon_ntff_profile_hook = lambda: _hook
        mod.set_axon_ntff_profile_hook = lambda h: None
        sys.modules["antenv.axon_hooks"] = mod
        antenv.axon_hooks = mod
    except Exception as e:  # profiling is optional; never break the run
        print(f"profiling shim unavailable: {e}")


def _build_program(T, TC):
    """Emit the Bass/Tile program for a T-step NLMS with TC-step chunks."""
    assert T % TC == 0
    nchunk = T // TC
    TCP = TC + L - 1  # chunk length incl. 31-sample lookback

    nc = bacc.Bacc(trn_type="TRN2")
    xp = nc.dram_tensor("xp", [128, nchunk, NBLK, TCP], F32, kind="ExternalInput")
    yt = nc.dram_tensor("yt", [128, nchunk, NBLK, TC], F32, kind="ExternalInput")
    rb = nc.dram_tensor("rb", [128, nchunk, NBLK, TC], F32, kind="ExternalInput")
    eo = nc.dram_tensor("eo", [128, nchunk, NBLK, TC], F32, kind="ExternalOutput")
    wo = nc.dram_tensor("wo", [128, NBLK, L], F32, kind="ExternalOutput")

    xp_ap, yt_ap, rb_ap, eo_ap, wo_ap = (
        t.ap() for t in (xp, yt, rb, eo, wo)
    )

    with ExitStack() as ctx:
        tc_ = ctx.enter_context(tile.TileContext(nc))
        wpool = ctx.enter_context(tc_.tile_pool(name="w", bufs=1))
        xpool = ctx.enter_context(tc_.tile_pool(name="x", bufs=2))
        ypool = ctx.enter_context(tc_.tile_pool(name="y", bufs=2))
        rpool = ctx.enter_context(tc_.tile_pool(name="r", bufs=2))
        epool = ctx.enter_context(tc_.tile_pool(name="e", bufs=2))
        ppool = ctx.enter_context(tc_.tile_pool(name="p", bufs=2))
        spool = ctx.enter_context(tc_.tile_pool(name="s", bufs=4))

        W = wpool.tile([128, NBLK, L], F32)
        nc.vector.memset(W, 0.0)

        for c in range(nchunk):
            xt = xpool.tile([128, NBLK, TCP], F32, tag="xt")
            nc.sync.dma_start(out=xt, in_=xp_ap[:, c])
            yc = ypool.tile([128, NBLK, TC], F32, tag="yc")
            nc.sync.dma_start(out=yc, in_=yt_ap[:, c])
            rc = rpool.tile([128, NBLK, TC], F32, tag="rc")
            nc.sync.dma_start(out=rc, in_=rb_ap[:, c])
            ec = epool.tile([128, NBLK, TC], F32, tag="ec")

            for k in range(TC):
                win = xt[:, :, k : k + L]
                P = ppool.tile([128, NBLK, L], F32, tag="P")
                nc.vector.tensor_tensor(P, W, win, op=mybir.AluOpType.mult)
                yh = spool.tile([128, NBLK], F32, tag="yh")
                nc.vector.tensor_reduce(
                    yh, P, axis=mybir.AxisListType.X, op=mybir.AluOpType.add
                )
                nc.vector.tensor_tensor(
                    ec[:, :, k], yc[:, :, k], yh, op=mybir.AluOpType.subtract
                )
                coef = spool.tile([128, NBLK], F32, tag="coef")
                nc.vector.tensor_tensor(
                    coef, ec[:, :, k], rc[:, :, k], op=mybir.AluOpType.mult
                )
                coef_b = coef.unsqueeze(-1).broadcast_to([128, NBLK, L])
                U = ppool.tile([128, NBLK, L], F32, tag="U")
                nc.vector.tensor_tensor(U, coef_b, win, op=mybir.AluOpType.mult)
                nc.vector.tensor_tensor(W, U, W, op=mybir.AluOpType.add)

            nc.sync.dma_start(out=eo_ap[:, c], in_=ec)

        nc.sync.dma_start(out=wo_ap, in_=W)

    nc.compile()
    return nc


def _get_program(T, TC):
    key = (T, TC)
    if key not in _PROGRAMS:
        _PROGRAMS[key] = _build_program(T, TC)
    return _PROGRAMS[key]


def _host_recb(Xc):
    """mu / (sliding window sum of squares + eps), fp64.  Xc: [CH, T]."""
    xsq = (Xc.astype(np.float64)) ** 2
    cs = np.cumsum(xsq, axis=1)
    n = cs.copy()
    n[:, L:] = cs[:, L:] - cs[:, :-L]
    return (MU / (n + EPS)).astype(np.float32)


def _pack_core(Xb, Yb, T, TC):
    """Pack one core's [BPC, T, F] inputs into device layouts.

    Returns xp [128, nchunk, NBLK, TC+31], yt [128, nchunk, NBLK, TC],
    rb likewise.  Channel ch = b*F + f lives at partition ch%128, block
    ch//128.
    """
    nchunk = T // TC
    TCP = TC + L - 1
    # [CH, T] channel-major
    Xc = np.zeros((CPAD, T), np.float32)
    Yc = np.zeros((CPAD, T), np.float32)
    Xc[:C] = Xb.transpose(0, 2, 1).reshape(C, T)
    Yc[:C] = Yb.transpose(0, 2, 1).reshape(C, T)
    Rc = np.zeros((CPAD, T), np.float32)
    Rc[:C] = _host_recb(Xc[:C])
    # zero-pad 31 front samples for the first windows
    Xp = np.concatenate([np.zeros((CPAD, L - 1), np.float32), Xc], axis=1)

    def dev(a):  # [CPAD, T*] -> [128, NBLK, T*]
        return np.ascontiguousarray(
            a.reshape(NBLK, 128, a.shape[1]).transpose(1, 0, 2)
        )

    Xd, Yd, Rd = dev(Xp), dev(Yc), dev(Rc)
    xp = np.empty((128, nchunk, NBLK, TCP), np.float32)
    yt = np.empty((128, nchunk, NBLK, TC), np.float32)
    rb = np.empty((128, nchunk, NBLK, TC), np.float32)
    for c in range(nchunk):
        xp[:, c] = Xd[:, :, c * TC : c * TC + TCP]
        yt[:, c] = Yd[:, :, c * TC : (c + 1) * TC]
        rb[:, c] = Rd[:, :, c * TC : (c + 1) * TC]
    return xp, yt, rb


def _unpack_core(e_dev, w_dev, T):
    """Inverse of _pack_core for the outputs of one core."""
    nchunk = e_dev.shape[1]
    TC = e_dev.shape[3]
    # e_dev [128, nchunk, NBLK, TC] -> [CPAD, T]
    ec = e_dev.transpose(2, 0, 1, 3).reshape(CPAD, T)
    E = ec[:C].reshape(BPC, F, T).transpose(0, 2, 1)
    wc = w_dev.transpose(1, 0, 2).reshape(CPAD, L)
    W = wc[:C].reshape(BPC, F, L).transpose(0, 2, 1)
    return E, W


def _run_device(X, Y, T, TC, trace=False):
    """X, Y: [B, T, F] float32.  Returns E [B, T, F], W [B, L, F]."""
    global LAST_RESULTS
    if trace:
        _install_profiling_shims()
    nc = _get_program(T, TC)
    in_maps = []
    for core in range(NCORES):
        Xb = X[core * BPC : (core + 1) * BPC]
        Yb = Y[core * BPC : (core + 1) * BPC]
        xp, yt, rb = _pack_core(Xb, Yb, T, TC)
        in_maps.append({"xp": xp, "yt": yt, "rb": rb})
    res = run_bass_kernel_spmd(
        nc, in_maps, core_ids=list(range(NCORES)), trace=trace
    )
    LAST_RESULTS = res
    E = np.empty((B, T, F), np.float32)
    W = np.empty((B, L, F), np.float32)
    for core in range(NCORES):
        Ec, Wc = _unpack_core(res.results[core]["eo"], res.results[core]["wo"], T)
        E[core * BPC : (core + 1) * BPC] = Ec
        W[core * BPC : (core + 1) * BPC] = Wc
    return E, W


def kernel(X_hat_mag, Y_mag, W_prev):
    X = np.asarray(X_hat_mag, np.float32)
    Y = np.asarray(Y_mag, np.float32)
    W0 = np.asarray(W_prev, np.float32)

    # The device path assumes W_prev == 0 (as produced by setup_inputs).
    # A nonzero W_prev only shifts y_hat by the fixed correlation
    # f0 = correlate(X, W_prev); feeding Y - f0 through the same recurrence
    # yields identical errors E, and W_final = W_dev + W_prev.
    if np.any(W0):
        Xp = np.pad(X, ((0, 0), (L - 1, 0), (0, 0)))
        win = np.lib.stride_tricks.sliding_window_view(Xp, L, axis=1)
        f0 = np.einsum("btfl,blf->btf", win, W0, optimize=True)
        Y = (Y - f0).astype(np.float32)

    trace = bool(int(os.environ.get("NLMS_TRACE", "0")))
    E, W = _run_device(X, Y, T_FULL, 250, trace=trace)
    if np.any(W0):
        W = W + W0
    return E, W


# revision 15
# speedup vs baseline: 1.0415x; 1.0415x over previous
"""Trainium2 Bass kernel for differentiable NLMS (nn_DifferentiableNLMS).

Problem: B=32 batches, T=2000 frames, F=513 freq bins, L=32 filter taps.
Per (batch, freq) channel an independent sequential NLMS recurrence runs
along time:
    y_hat_k = sum_l W[l] * x[k-L+1+l]
    e_k     = y_k - y_hat_k
    n_k     = sum_l x[k-L+1+l]^2 + eps
    W      += mu * (e_k / n_k) * window_k

Sharding: batch dim across 8 cores (4 batches/core -> 2052 channels/core,
padded to 17*128).  Channels live on partitions (128) x free-dim blocks (17).
The norm reciprocals mu/n_k depend only on X and are precomputed host-side
in fp64 (cumsum sliding sums); the device executes the sequential recurrence
with 6 vector-engine ops per time step, reading the sliding window directly
out of the chunk-resident X tile via strided access patterns.
"""

import os
from contextlib import ExitStack

import numpy as np

import concourse.bacc as bacc
import concourse.bass as bass
import concourse.tile as tile
from concourse import mybir
from concourse.bass_utils import run_bass_kernel_spmd

L = 32
MU = 0.1
EPS = 1e-8
B, T_FULL, F = 32, 2000, 513
NCORES = 8
BPC = B // NCORES            # batches per core
C = BPC * F                  # 2052 channels per core
NBLK = 16                    # free-dim blocks on device
CDEV = NBLK * 128            # 2048 channels on device per core
# leftover C - CDEV = 4 channels/core (batch b_local=3, freqs 509..512)
# are evaluated host-side in _nlms_host_channels.
F32 = mybir.dt.float32

# module-level cache: (T, TC) -> (nc, names)
_PROGRAMS = {}
LAST_RESULTS = None


def _install_profiling_shims():
    """Best-effort NTFF profiling support under axon (missing antenv.axon_hooks)."""
    try:
        import contextlib
        import ctypes
        import sys
        import types

        import antenv  # noqa: F401
        import concourse.bass_utils as bu

        if not getattr(bu.upload_artifacts, "_nlms_patched", False):
            def _local_upload(tmpdir):
                return "local://" + str(tmpdir)

            _local_upload._nlms_patched = True
            bu.upload_artifacts = _local_upload

        try:
            import antenv.axon_hooks  # noqa: F401
            return
        except ImportError:
            pass

        so_path = os.environ.get("PJRT_LIBRARY_PATH", "/opt/axon/libaxon_pjrt.so")
        lib = ctypes.CDLL(so_path)
        if not hasattr(lib, "axon_start_nrt_profile"):
            return
        lib.axon_start_nrt_profile.argtypes = [
            ctypes.POINTER(ctypes.c_int64),
            ctypes.c_size_t,
        ]
        lib.axon_start_nrt_profile.restype = ctypes.c_int64
        lib.axon_stop_nrt_profile.argtypes = [ctypes.c_char_p]
        lib.axon_stop_nrt_profile.restype = ctypes.c_int64

        @contextlib.contextmanager
        def _hook(output_dir, device_ids):
            import jax

            jax.devices()
            if device_ids:
                ids = (ctypes.c_int64 * len(device_ids))(*device_ids)
                rc = lib.axon_start_nrt_profile(ids, len(device_ids))
            else:
                rc = lib.axon_start_nrt_profile(None, 0)
            if rc != 0:
                raise RuntimeError(f"axon_start_nrt_profile rc={rc}")
            try:
                yield
            finally:
                n = lib.axon_stop_nrt_profile(str(output_dir).encode())
                print(f"profile: {n} file(s) written to {output_dir}")

        mod = types.ModuleType("antenv.axon_hooks")
        mod.get_axon_ntff_profile_hook = lambda: _hook
        mod.set_axon_ntff_profile_hook = lambda h: None
        sys.modules["antenv.axon_hooks"] = mod
        antenv.axon_hooks = mod
    except Exception as e:  # profiling is optional; never break the run
        print(f"profiling shim unavailable: {e}")


def _build_program(T, TC):
    """Emit the Bass/Tile program for a T-step NLMS with TC-step chunks.

    Two time steps are processed per round: predictions for steps k and
    k+1 are batched against W_k (one mul + one reduce over [blk, 2, L]),
    and the step-(k+1) error is then corrected exactly with the lag-1
    window Gram r1(k) = <win_k, win_{k+1}> (host precomputed):
        e_{k+1} = y_{k+1} - W_k.win_{k+1} - coef_k * r1(k).
    """
    assert T % TC == 0 and TC % 2 == 0
    nchunk = T // TC
    TCP = TC + L - 1  # chunk length incl. 31-sample lookback

    nc = bacc.Bacc(trn_type="TRN2")
    xp = nc.dram_tensor("xp", [128, nchunk, NBLK, TCP], F32, kind="ExternalInput")
    yt = nc.dram_tensor("yt", [128, nchunk, NBLK, TC], F32, kind="ExternalInput")
    rb = nc.dram_tensor("rb", [128, nchunk, NBLK, TC], F32, kind="ExternalInput")
    r1 = nc.dram_tensor("r1", [128, nchunk, NBLK, TC], F32, kind="ExternalInput")
    eo = nc.dram_tensor("eo", [128, nchunk, NBLK, TC], F32, kind="ExternalOutput")
    wo = nc.dram_tensor("wo", [128, NBLK, L], F32, kind="ExternalOutput")

    xp_ap, yt_ap, rb_ap, r1_ap, eo_ap, wo_ap = (
        t.ap() for t in (xp, yt, rb, r1, eo, wo)
    )

    with ExitStack() as ctx:
        tc_ = ctx.enter_context(tile.TileContext(nc))
        wpool = ctx.enter_context(tc_.tile_pool(name="w", bufs=1))
        xpool = ctx.enter_context(tc_.tile_pool(name="x", bufs=2))
        ypool = ctx.enter_context(tc_.tile_pool(name="y", bufs=2))
        rpool = ctx.enter_context(tc_.tile_pool(name="r", bufs=2))
        r1pool = ctx.enter_context(tc_.tile_pool(name="r1", bufs=2))
        epool = ctx.enter_context(tc_.tile_pool(name="e", bufs=2))
        ppool = ctx.enter_context(tc_.tile_pool(name="p", bufs=2))
        spool = ctx.enter_context(tc_.tile_pool(name="s", bufs=4))

        W = wpool.tile([128, NBLK, L], F32)
        nc.vector.memset(W, 0.0)
        mult, add, sub = (
            mybir.AluOpType.mult,
            mybir.AluOpType.add,
            mybir.AluOpType.subtract,
        )

        for c in range(nchunk):
            xt = xpool.tile([128, NBLK, TCP], F32, tag="xt")
            nc.sync.dma_start(out=xt, in_=xp_ap[:, c])
            yc = ypool.tile([128, NBLK, TC], F32, tag="yc")
            nc.sync.dma_start(out=yc, in_=yt_ap[:, c])
            rc = rpool.tile([128, NBLK, TC], F32, tag="rc")
            nc.sync.dma_start(out=rc, in_=rb_ap[:, c])
            r1c = r1pool.tile([128, NBLK, TC], F32, tag="r1c")
            nc.sync.dma_start(out=r1c, in_=r1_ap[:, c])
            ec = epool.tile([128, NBLK, TC], F32, tag="ec")

            for k in range(0, TC, 2):
                # overlapping windows for steps k, k+1: [128, NBLK, 2, L]
                win2 = bass.AP(
                    tensor=xt.tensor,
                    offset=xt.offset + k,
                    ap=[[NBLK * TCP, 128], [TCP, NBLK], [1, 2], [1, L]],
                )
                Wb = W.unsqueeze(2).broadcast_to([128, NBLK, 2, L])
                P2 = ppool.tile([128, NBLK, 2, L], F32, tag="P")
                nc.vector.tensor_tensor(P2, Wb, win2, op=mult)
                yh2 = spool.tile([128, NBLK, 2], F32, tag="yh")
                nc.vector.tensor_reduce(
                    yh2, P2, axis=mybir.AxisListType.X, op=add
                )
                # raw errors for both lanes (lane k exact; lane k+1 pending fix)
                nc.vector.tensor_tensor(
                    ec[:, :, k : k + 2], yc[:, :, k : k + 2], yh2, op=sub
                )
                cf2 = spool.tile([128, NBLK, 2], F32, tag="cf")
                nc.vector.tensor_tensor(
                    cf2[:, :, 0], ec[:, :, k], rc[:, :, k], op=mult
                )
                t = spool.tile([128, NBLK], F32, tag="t")
                nc.vector.tensor_tensor(t, cf2[:, :, 0], r1c[:, :, k], op=mult)
                nc.vector.tensor_tensor(
                    ec[:, :, k + 1], ec[:, :, k + 1], t, op=sub
                )
                nc.vector.tensor_tensor(
                    cf2[:, :, 1], ec[:, :, k + 1], rc[:, :, k + 1], op=mult
                )
                cb2 = cf2.unsqueeze(-1).broadcast_to([128, NBLK, 2, L])
                U2 = ppool.tile([128, NBLK, 2, L], F32, tag="U")
                nc.vector.tensor_tensor(U2, cb2, win2, op=mult)
                nc.vector.tensor_tensor(W, U2[:, :, 0, :], W, op=add)
                nc.vector.tensor_tensor(W, U2[:, :, 1, :], W, op=add)

            nc.sync.dma_start(out=eo_ap[:, c], in_=ec)

        nc.sync.dma_start(out=wo_ap, in_=W)

    nc.compile()
    return nc


def _get_program(T, TC):
    key = (T, TC)
    if key not in _PROGRAMS:
        _PROGRAMS[key] = _build_program(T, TC)
    return _PROGRAMS[key]


def _host_recb(Xc):
    """mu / (sliding window sum of squares + eps), fp64.  Xc: [CH, T]."""
    xsq = (Xc.astype(np.float64)) ** 2
    cs = np.cumsum(xsq, axis=1)
    n = cs.copy()
    n[:, L:] = cs[:, L:] - cs[:, :-L]
    return (MU / (n + EPS)).astype(np.float32)


def _host_r1(Xc):
    """Lag-1 window Gram r1(k) = <win_k, win_{k+1}>, fp64.  Xc: [CH, T]."""
    x = Xc.astype(np.float64)
    q1 = np.zeros_like(x)
    q1[:, :-1] = x[:, :-1] * x[:, 1:]
    cs = np.cumsum(q1, axis=1)
    r = cs.copy()
    r[:, L:] = cs[:, L:] - cs[:, :-L]
    return r.astype(np.float32)


def _pack_core(Xb, Yb, T, TC):
    """Pack one core's [BPC, T, F] inputs into device layouts.

    Returns xp [128, nchunk, NBLK, TC+31], yt [128, nchunk, NBLK, TC],
    rb likewise.  Channel ch = b*F + f (first CDEV only) lives at
    partition ch%128, block ch//128.
    """
    nchunk = T // TC
    TCP = TC + L - 1
    # [CH, T] channel-major, device channels only
    Xc = Xb.transpose(0, 2, 1).reshape(C, T)[:CDEV]
    Yc = np.ascontiguousarray(Yb.transpose(0, 2, 1).reshape(C, T)[:CDEV])
    Rc = _host_recb(Xc)
    R1c = _host_r1(Xc)
    # zero-pad 31 front samples for the first windows
    Xp = np.concatenate([np.zeros((CDEV, L - 1), np.float32), Xc], axis=1)

    def dev(a):  # [CDEV, T*] -> [128, NBLK, T*]
        return np.ascontiguousarray(
            a.reshape(NBLK, 128, a.shape[1]).transpose(1, 0, 2)
        )

    Xd, Yd, Rd, R1d = dev(Xp), dev(Yc), dev(Rc), dev(R1c)
    xp = np.empty((128, nchunk, NBLK, TCP), np.float32)
    yt = np.empty((128, nchunk, NBLK, TC), np.float32)
    rb = np.empty((128, nchunk, NBLK, TC), np.float32)
    r1 = np.empty((128, nchunk, NBLK, TC), np.float32)
    for c in range(nchunk):
        xp[:, c] = Xd[:, :, c * TC : c * TC + TCP]
        yt[:, c] = Yd[:, :, c * TC : (c + 1) * TC]
        rb[:, c] = Rd[:, :, c * TC : (c + 1) * TC]
        r1[:, c] = R1d[:, :, c * TC : (c + 1) * TC]
    return xp, yt, rb, r1


def _unpack_core(e_dev, w_dev, T):
    """Inverse of _pack_core; returns channel-major [CDEV, T] / [CDEV, L]."""
    ec = e_dev.transpose(2, 0, 1, 3).reshape(CDEV, T)
    wc = w_dev.transpose(1, 0, 2).reshape(CDEV, L)
    return ec, wc


def _nlms_host_channels(x, y):
    """Exact NLMS for a small set of channels.  x, y: [N, T] -> e, w."""
    N, T = x.shape
    xp = np.concatenate([np.zeros((N, L - 1), np.float32), x], axis=1)
    rec = _host_recb(x)
    W = np.zeros((N, L), np.float32)
    e = np.empty((N, T), np.float32)
    for k in range(T):
        win = xp[:, k : k + L]
        yh = np.einsum("nl,nl->n", W, win)
        ek = y[:, k] - yh
        e[:, k] = ek
        W += (ek * rec[:, k])[:, None] * win
    return e, W


def _run_device(X, Y, T, TC, trace=False):
    """X, Y: [B, T, F] float32.  Returns E [B, T, F], W [B, L, F]."""
    global LAST_RESULTS
    if trace:
        _install_profiling_shims()
    nc = _get_program(T, TC)
    in_maps = []
    for core in range(NCORES):
        Xb = X[core * BPC : (core + 1) * BPC]
        Yb = Y[core * BPC : (core + 1) * BPC]
        xp, yt, rb, r1 = _pack_core(Xb, Yb, T, TC)
        in_maps.append({"xp": xp, "yt": yt, "rb": rb, "r1": r1})
    res = run_bass_kernel_spmd(
        nc, in_maps, core_ids=list(range(NCORES)), trace=trace
    )
    LAST_RESULTS = res

    # leftover channels (per core: b_local=3, f>=CDEV-3*F) on host
    FCUT = CDEV - 3 * F  # 509
    xl = X[3::4, :, FCUT:].transpose(0, 2, 1).reshape(-1, T)  # [8*(F-FCUT), T]
    yl = np.ascontiguousarray(Y[3::4, :, FCUT:].transpose(0, 2, 1).reshape(-1, T))
    el, wl = _nlms_host_channels(np.ascontiguousarray(xl), yl)
    nleft = F - FCUT

    E = np.empty((B, T, F), np.float32)
    W = np.empty((B, L, F), np.float32)
    for core in range(NCORES):
        ec, wc = _unpack_core(res.results[core]["eo"], res.results[core]["wo"], T)
        b0 = core * BPC
        # device channels 0..CDEV-1 = batches 0..2 full + batch 3 freqs < FCUT
        E[b0 : b0 + 3] = ec[: 3 * F].reshape(3, F, T).transpose(0, 2, 1)
        W[b0 : b0 + 3] = wc[: 3 * F].reshape(3, F, L).transpose(0, 2, 1)
        E[b0 + 3, :, :FCUT] = ec[3 * F :].T
        W[b0 + 3, :, :FCUT] = wc[3 * F :].T
        E[b0 + 3, :, FCUT:] = el[core * nleft : (core + 1) * nleft].T
        W[b0 + 3, :, FCUT:] = wl[core * nleft : (core + 1) * nleft].T
    return E, W


def kernel(X_hat_mag, Y_mag, W_prev):
    X = np.asarray(X_hat_mag, np.float32)
    Y = np.asarray(Y_mag, np.float32)
    W0 = np.asarray(W_prev, np.float32)

    # The device path assumes W_prev == 0 (as produced by setup_inputs).
    # A nonzero W_prev only shifts y_hat by the fixed correlation
    # f0 = correlate(X, W_prev); feeding Y - f0 through the same recurrence
    # yields identical errors E, and W_final = W_dev + W_prev.
    if np.any(W0):
        Xp = np.pad(X, ((0, 0), (L - 1, 0), (0, 0)))
        win = np.lib.stride_tricks.sliding_window_view(Xp, L, axis=1)
        f0 = np.einsum("btfl,blf->btf", win, W0, optimize=True)
        Y = (Y - f0).astype(np.float32)

    trace = bool(int(os.environ.get("NLMS_TRACE", "0")))
    E, W = _run_device(X, Y, T_FULL, 200, trace=trace)
    if np.any(W0):
        W = W + W0
    return E, W
